# revision 24
# baseline (speedup 1.0000x reference)
"""Trainium2 Bass kernel for nn_BaselineTargetHead (per-sample dynamic MLP).

Strategy: data-parallel over 8 NeuronCores, 8 samples per core.
Per sample the chain is 5 per-sample linear layers over 64 spatial positions:
  [1024,2048] @ [2048,64] -> sigmoid -> ... -> [1,128] @ [128,64] + b

Every weight element is used exactly once, so the kernel is HBM-stream bound:
~23 MB/core of fp8 weights at ~360 GB/s. The compute is classic stationary-
weight matmuls (lhsT = W^T tile [128,128] fp8, rhs = activation tile [128,64]
fp16); with FWL the LDWEIGHTS fully hides behind the matmul stream, so the PE
(~55 us/core) stays underneath the DMA stream (~65 us). ScalarE applies
scale+bias+sigmoid fused from PSUM.

Key structural points:
  - weights travel as fp8 e3m4 scaled by 64 (1/64 folded into the activation
    free affine). fc5's weights stay fp16: their quantization error hits the
    output with no sigmoid attenuation (fp8 there alone costs ~2% rel err).
  - one packed per-sample slab [x | L1 m-major | L2 | L3 | L4]; two DMA
    chunks per sample (9 KB / 13 KB partition lines keep the SDMA packets
    big), five finer chunks for sample 0 so the PE starts early.
  - samples are software-pipelined depth 1: the PE stream is
    [L1(s0)], [L1(s1), L2-5(s0)], [L1(s2), L2-5(s1)], ... so every
    layer-boundary sigmoid has a full L1 block of slack to complete.
"""

import numpy as np
import ml_dtypes

import concourse.bass as bass
import concourse.mybir as mybir
import concourse.tile as tile
from concourse.bass_utils import run_bass_kernel_spmd

N_CORES = 8
B = 64
S_PER_CORE = B // N_CORES  # 8 samples per core
HW = 64  # 8x8 spatial positions
DIMS = [2048, 1024, 512, 256, 128, 1]
LAYERS = [(2048, 1024), (1024, 512), (512, 256), (256, 128)]  # (Cin, Cout) of fc1..fc4
W_SCALE_FP8 = 64.0  # lift weights into e3m4's normal range; undone in the act scale
FP8_CLIP = 15.0  # e3m4 saturates to inf above 15.5

X_COLS = (2048 // 128) * HW  # 1024
W5_COLS = 32  # w5 in col 0, zero-padded to 32 cols for a legal M=32 matmul
L_COLS = [(ci // 128) * co for ci, co in LAYERS]  # 16384, 4096, 1024, 256
# slab column map: [x | L1 (m-major) | L2 (m-major) | L3 | L4]
L1_OFF = X_COLS  # 1024
L2_OFF = L1_OFF + L_COLS[0]  # 17408
L3_OFF = L2_OFF + L_COLS[1]  # 21504
L4_OFF = L3_OFF + L_COLS[2]  # 22528
TOT_COLS = L4_OFF + L_COLS[3]  # 22784
# bias image columns per sample: fc1 m0..7 | fc2 m0..3 | fc3 m0..1 | fc4 m0 | fc5
BIAS_COL0 = [0, 8, 12, 14]
BIAS_COLS = 16
# chunk tables: one whole-slab DMA per steady sample (22.3 KB partition
# lines maximize SDMA packet size; measured 382 GB/s at 17 KB lines vs 339
# at 9-13 KB). Sample 0 is split five ways so the PE starts early; sample 7
# in two so its compute overlaps its own stream tail.
CHUNKS = [0, TOT_COLS]
CHUNKS_S0 = [0, 5120, 9216, 13312, 17408, TOT_COLS]
CHUNKS_S7 = [0, 9216, TOT_COLS]

def _split_ctrl_multiwaits(nc):
    """walrus in this env rejects >1 sync-wait per instruction. Move extra
    waits onto NOPs placed immediately before, on the same engine — engines
    execute in order, so this is semantically identical."""
    n_fixed = 0
    for bb in nc.main_func.blocks:
        insts = bb.instructions
        i = 0
        while i < len(insts):
            ins = insts[i]
            si = ins.sync_info
            if si is not None and si.on_wait and len(si.on_wait) > 1:
                waits = list(si.on_wait)
                new_nops = []
                for j, w in enumerate(waits[1:]):
                    nop = mybir.InstNoOp(name=f"{ins.name}-splitw-{j}", ins=[], outs=[])
                    nop.engine = ins.engine
                    nop.sync_info = mybir.SyncInfo(on_update=[], on_wait=[w])
                    new_nops.append(nop)
                si.on_wait = [waits[0]]
                insts[i:i] = new_nops
                i += len(new_nops)
                n_fixed += 1
            i += 1
    return n_fixed


def _build_nc():
    f8 = mybir.dt.float8e3
    f16 = mybir.dt.float16
    f32 = mybir.dt.float32
    nc = bass.Bass()
    slab_d = nc.dram_tensor("slab", [S_PER_CORE, 128, TOT_COLS], f8, kind="ExternalInput")
    w5_d = nc.dram_tensor("w5", [128, S_PER_CORE * W5_COLS], f16, kind="ExternalInput")
    bias_d = nc.dram_tensor("bias", [128, S_PER_CORE * BIAS_COLS], f32, kind="ExternalInput")
    out_d = nc.dram_tensor("out", [S_PER_CORE, HW], f32, kind="ExternalOutput")

    sig = mybir.ActivationFunctionType.Sigmoid
    ident_fn = mybir.ActivationFunctionType.Identity
    inv_s = 1.0 / W_SCALE_FP8

    with tile.TileContext(nc) as tc:
        with (
            tc.tile_pool(name="wpool", bufs=4) as wpool,
            tc.tile_pool(name="qpool", bufs=2) as qpool,
            tc.tile_pool(name="misc", bufs=1) as misc,
            tc.tile_pool(name="psum", bufs=1, space="PSUM") as psum_pool,
        ):
            bias_sb = misc.tile([128, S_PER_CORE * BIAS_COLS], f32)
            nc.scalar.dma_start(bias_sb[:], bias_d[:])
            w5_sb = misc.tile([128, S_PER_CORE * W5_COLS], f16)
            nc.scalar.dma_start(w5_sb[:], w5_d[:])
            collect = misc.tile([1, S_PER_CORE * HW], f32)

            # per-sample chunk tiles, keyed by the chunk table
            chunk_tiles = [None] * S_PER_CORE

            def stage_dma(s):
                if s == 0:
                    table, pool, tag = CHUNKS_S0, misc, None
                elif s == S_PER_CORE - 1:
                    table, pool, tag = CHUNKS_S7, misc, None
                else:
                    table, pool, tag = CHUNKS, wpool, "slab"
                tiles = []
                for i in range(len(table) - 1):
                    kw = dict(tag=tag) if tag else {}
                    t = pool.tile([128, table[i + 1] - table[i]], f8,
                                  name=f"s{s}c{i}", **kw)
                    nc.sync.dma_start(
                        t[:], slab_d[s, :, table[i] : table[i + 1]])
                    tiles.append(t)
                chunk_tiles[s] = (table, tiles)

            def slab_slice(s, col0, ncols):
                table, tiles = chunk_tiles[s]
                i = 0
                while table[i + 1] <= col0:
                    i += 1
                assert col0 + ncols <= table[i + 1], (s, col0, ncols)
                return tiles[i][:, col0 - table[i] : col0 - table[i] + ncols]

            def stage_l1(s):
                q1 = qpool.tile([128, 8 * HW], f16, tag="q0", name=f"q1_{s}")
                kt = 2048 // 128
                for m in range(8):
                    ps = psum_pool.tile([128, HW], f32, tag="ps", bufs=8,
                                        name=f"psA_{s}_{m}")
                    for k in range(kt):
                        lhsT = slab_slice(s, L1_OFF + (m * kt + k) * 128, 128)
                        rhs = slab_slice(s, k * HW, HW)
                        nc.tensor.matmul(
                            ps[:], lhsT, rhs, start=(k == 0), stop=(k == kt - 1)
                        )
                    nc.scalar.activation(
                        q1[:, m * HW : (m + 1) * HW],
                        ps[:],
                        sig,
                        bias=bias_sb[:, s * BIAS_COLS + m : s * BIAS_COLS + m + 1],
                        scale=inv_s,
                    )
                return q1

            def stage_tail(s, q1):
                q_prev = q1[:]
                for li, (cin, cout) in enumerate(LAYERS[1:], start=1):
                    kt, mt = cin // 128, cout // 128
                    off = (L2_OFF, L3_OFF, L4_OFF)[li - 1]
                    qn = qpool.tile([128, mt * HW], f16, tag=f"q{li}",
                                    name=f"q{li}_{s}")
                    for m in range(mt):
                        ps = psum_pool.tile([128, HW], f32, tag="ps", bufs=8,
                                            name=f"psB{li}_{s}_{m}")
                        for k in range(kt):
                            lhsT = slab_slice(s, off + (m * kt + k) * 128, 128)
                            nc.tensor.matmul(
                                ps[:], lhsT, q_prev[:, k * HW : (k + 1) * HW],
                                start=(k == 0), stop=(k == kt - 1),
                            )
                        bcol = s * BIAS_COLS + BIAS_COL0[li] + m
                        nc.scalar.activation(
                            qn[:, m * HW : (m + 1) * HW],
                            ps[:],
                            sig,
                            bias=bias_sb[:, bcol : bcol + 1],
                            scale=inv_s,
                        )
                    q_prev = qn[:]

                ps5 = psum_pool.tile([128, HW], f32, tag="ps", bufs=8,
                                     name=f"ps5_{s}")
                nc.tensor.matmul(
                    ps5[0:32, :], w5_sb[:, s * W5_COLS : (s + 1) * W5_COLS],
                    q_prev[:, 0:HW], start=True, stop=True,
                )
                b5col = s * BIAS_COLS + 15
                nc.scalar.activation(
                    collect[0:1, s * HW : (s + 1) * HW],
                    ps5[0:1, :],
                    ident_fn,
                    bias=bias_sb[0:1, b5col : b5col + 1],
                    scale=1.0,
                )

            q1s = [None] * S_PER_CORE
            for i in range(S_PER_CORE + 1):
                if i < S_PER_CORE:
                    stage_dma(i)
                    q1s[i] = stage_l1(i)
                if i >= 1:
                    stage_tail(i - 1, q1s[i - 1])
            nc.scalar.dma_start(out_d[:], collect[:])

    _split_ctrl_multiwaits(nc)
    return nc


_NC_CACHE = None


def _get_nc():
    global _NC_CACHE
    if _NC_CACHE is None:
        _NC_CACHE = _build_nc()
    return _NC_CACHE


def _to_fp8(a):
    return np.clip(a, -FP8_CLIP, FP8_CLIP).astype(ml_dtypes.float8_e3m4)


def _prep_core(inputs, c):
    """Build the per-core input map (numpy only, host-side layout prep)."""
    sl = slice(c * S_PER_CORE, (c + 1) * S_PER_CORE)

    # x image: [S, 128, 1024] with img[s, p, k*64+h] = x[s, k*128+p, h]
    x = inputs["target_in_vec"][sl].reshape(S_PER_CORE, 2048 // 128, 128, HW)
    ximg = _to_fp8(x.transpose(0, 2, 1, 3).reshape(S_PER_CORE, 128, X_COLS))
    w5pad = np.zeros((S_PER_CORE, 128, W5_COLS), np.float16)
    w5pad[:, :, 0] = inputs["target_fc5w"][sl, 0, :, 0, 0]  # [S, 128]
    w5img = np.ascontiguousarray(
        w5pad.transpose(1, 0, 2).reshape(128, S_PER_CORE * W5_COLS)
    )

    # all layers m-block-major (stationary):
    # img[s, p, (m*kt+k)*128 + c] = w[s, m*128+c, k*128+p] * 64
    wparts = [ximg]
    for li, (cin, cout) in enumerate(LAYERS):
        kt, mt = cin // 128, cout // 128
        w = inputs[f"target_fc{li + 1}w"][sl, :, :, 0, 0]  # [S, Cout, Cin]
        wt = w.reshape(S_PER_CORE, mt, 128, kt, 128)  # [s, m, c, k, p]
        wt = wt.transpose(0, 4, 1, 3, 2).reshape(S_PER_CORE, 128, kt * mt * 128)
        wparts.append(_to_fp8(wt * W_SCALE_FP8))
    slab = np.ascontiguousarray(np.concatenate(wparts, axis=2))
    assert slab.shape[2] == TOT_COLS

    bias = np.zeros((S_PER_CORE, 128, BIAS_COLS), np.float32)
    for li, (cin, cout) in enumerate(LAYERS):
        b = inputs[f"target_fc{li + 1}b"][sl]  # [S, Cout]
        bias[:, :, BIAS_COL0[li] : BIAS_COL0[li] + cout // 128] = b.reshape(
            S_PER_CORE, cout // 128, 128
        ).transpose(0, 2, 1)
    bias[:, 0, 15] = inputs["target_fc5b"][sl, 0]
    bias = np.ascontiguousarray(bias.transpose(1, 0, 2).reshape(128, -1))

    return {"slab": slab, "w5": w5img, "bias": bias}


def kernel(**inputs):
    inputs = {k: np.asarray(v) for k, v in inputs.items()}
    nc = _get_nc()
    in_maps = [_prep_core(inputs, c) for c in range(N_CORES)]
    res = run_bass_kernel_spmd(nc, in_maps, list(range(N_CORES)))
    out = np.concatenate([np.asarray(res.results[c]["out"]) for c in range(N_CORES)], axis=0)
    return out.reshape(B, 8, 8).astype(np.float32)
